# revision 1
# baseline (speedup 1.0000x reference)
"""Causal single-head attention on 8 Trainium2 NeuronCores (Bass/Tile).

Problem: x[4,2048,1024] fp32, Wq/Wk/Wv[1024,1024];
  q,k,v = x@W.T ; S = q@k.T/sqrt(d) ; causal softmax ; out = P@v.

Sharding: core c -> (batch b=c//2, h=c%2). Balanced causal split: core h
owns global 256-row chunks GMAP[h] (h=0: [6,7,0,1], h=1: [4,5,2,3]),
ascending within each 512-row window so window-local addressing stays
affine. Both cores of a pair do equal causal work. K/V halves are
exchanged with a pairwise AllGather; the gathered buffer is identical on
both cores, so the permuted readback into natural key order is
SPMD-uniform.

Scores are computed transposed, S^T[k, q], 512 queries per window, only
key blocks 0..c_w-1 (causal skipping, padded to the max of the two
cores' needs; pad is masked). Softmax skips max-subtraction (scores are
O(1) here; exp is fp32-safe); the mask is additive before exp via two
affine_selects per diagonal block whose fills come from two per-core
scalars. P^T feeds AV directly as the stationary operand (keys on
partitions on both sides -- no PE transposes); Z rides along as a
1-wide ones-matmul into the same accumulation group structure. AV runs
per 256-row half-slot with its own tighter causal limit.

All tensors are merged into wide SBUF tiles ([128, segments*width]) and
inputs are host-pre-swizzled to [128, 8192] so every load/spill/readback
is one or a few large DMAs (DMA instruction dispatch is a serialized
resource). bf16 data path (PE is 1 cycle/row for bf16, same as fp32r),
fp32 PSUM/statistics/output. Rel err ~5e-3 vs the fp32 reference.

DMA queues: SP carries pure input loads, Activation carries spills and
output stores, Pool/SWDGE carries the exchange chain (emulated-
collective copies + readbacks) so dependency waits never head-of-line
block input prefetch.
"""

import sys

sys.path.insert(0, "/opt/trn_rl_repo")

from contextlib import ExitStack

import numpy as np

import concourse.bass as bass
from concourse import bacc
import concourse.mybir as mybir
import concourse.tile as tile
from concourse.bass_utils import run_bass_kernel_spmd

F32 = mybir.dt.float32
BF16 = mybir.dt.bfloat16

B, N, D = 4, 2048, 1024
P = 128
NQ = N // 2      # local rows per core
ND = D // P      # 8 d-blocks
NO = D // P      # 8 o-blocks
MASK_VAL = -1.0e30
GROUPS = [[0, 1], [2, 3], [4, 5], [6, 7]]

# local 256-chunk s (slot) -> global 256-chunk; chosen so slot pairs are
# globally adjacent ({7,6},{5,4},{2,3},{0,1}) and position-wise-max padding
# is minimal ({16,12,6,2} | {14,10,8,4})
GMAP = [[7, 5, 2, 0], [6, 4, 3, 1]]
S_C = [16, 12, 8, 4]       # score key-blocks per slot (max over pair)
S_GLO = [6, 4, 2, 0]       # low-core global chunk per slot
S_GHI = [7, 5, 3, 1]       # high-core global chunk per slot
S_MCOL = [0, 0, 1, 1]      # which m1 column rules each slot
QS_C = [[15, 16], [11, 12], [7, 8], [3, 4]]  # AV limit per (slot, 128-qsub)

_CACHE = {}


def _build_program(iters=1, phase="full"):
    nc = bacc.Bacc("TRN2", target_bir_lowering=False, debug=False, num_devices=8)
    xT = nc.dram_tensor("xT", [P, ND * NQ], BF16, kind="ExternalInput").ap()
    wqT = nc.dram_tensor("wqT", [P, ND * D], BF16, kind="ExternalInput").ap()
    wkT = nc.dram_tensor("wkT", [P, ND * D], BF16, kind="ExternalInput").ap()
    wvT = nc.dram_tensor("wvT", [P, ND * D], BF16, kind="ExternalInput").ap()
    m1 = nc.dram_tensor("m1", [P, 2], F32, kind="ExternalInput").ap()
    out = nc.dram_tensor("out", [P, 8 * D], F32, kind="ExternalOutput").ap()

    with tile.TileContext(nc) as tc:
        if iters == 1:
            _attention_kernel(tc, out, xT, wqT, wkT, wvT, m1, phase)
        else:
            # unroll inside the hw loop: For_i puts an all-engine barrier at
            # each back edge; consecutive unrolled bodies overlap via pool-zone
            # release syncs instead, so the barrier cost amortizes.
            unroll = next(u for u in (4, 2, 1) if iters % u == 0)
            with tc.For_i(0, iters // unroll, 1):
                for _ in range(unroll):
                    _attention_kernel(tc, out, xT, wqT, wkT, wvT, m1, phase)
    nc.compile()
    return nc


def _attention_kernel(tc, out, xT, wqT, wkT, wvT, m1, phase="full"):
    nc = tc.nc

    with ExitStack() as ctx:
        # ---- per-core mask scalars / small constants ----
        const_pool = ctx.enter_context(tc.tile_pool(name="const", bufs=1))
        m1_sb = const_pool.tile([P, 2], F32, tag="m1")
        nc.sync.dma_start(m1_sb[:], m1[:, :])
        ones = const_pool.tile([P, 1], BF16, tag="ones")
        nc.vector.memset(ones[:], 1.0)
        zeros = const_pool.tile([P, 256], F32, tag="zeros")
        nc.vector.memset(zeros[:], 0.0)
        m1row = [const_pool.tile([P, 256], F32, tag=f"m1r{i}", name=f"m1r{i}")
                 for i in range(2)]
        for i in range(2):
            nc.vector.tensor_scalar_add(m1row[i][:], zeros[:], m1_sb[:, i : i + 1])
        mreg = getattr(nc, "_mreg_cache", None)
        if mreg is None:
            mreg = [nc.gpsimd.alloc_register(f"mreg{i}") for i in range(2)]
            nc._mreg_cache = mreg
        for i in range(2):
            nc.gpsimd.reg_load(mreg[i], m1_sb[0:1, i : i + 1].bitcast(mybir.dt.int32))

        # DRAM bounce buffers for the pairwise K/V all-gathers
        dram_pool = ctx.enter_context(tc.tile_pool(name="dram", bufs=1, space="DRAM"))
        k_own_d = dram_pool.tile([P, ND * NQ], BF16, tag="k_own")  # [p, ob*1024+lk]
        v_own_d = dram_pool.tile([P, 8 * D], BF16, tag="v_own")    # [p, sb*1024+o]
        k_g = dram_pool.tile([2 * P, ND * NQ], BF16, tag="k_g")
        v_g = dram_pool.tile([2 * P, 8 * D], BF16, tag="v_g")

        # resident attention operands (merged wide tiles)
        big_pool = ctx.enter_context(tc.tile_pool(name="big", bufs=1))
        KTb = big_pool.tile([P, NO * N], BF16, tag="ktb")     # [p, ob*2048+key]
        Vb = big_pool.tile([P, 16 * D], BF16, tag="vb")       # [p, kb*1024+o]
        QTb = big_pool.tile([P, NO * NQ], BF16, tag="qtb")    # [p, ob*1024+lq]

        # attention SBUF pools come first: they must not re-use the proj pools'
        # zones, so the next body's input loads only wait on proj-zone release
        pt_pool = ctx.enter_context(tc.tile_pool(name="pt", bufs=1))
        mask_pool = ctx.enter_context(tc.tile_pool(name="mask", bufs=2))
        stat_pool = ctx.enter_context(tc.tile_pool(name="stat", bufs=4))
        o_pool = ctx.enter_context(tc.tile_pool(name="o", bufs=2))

        # ================= projections (own 1024 rows only) =================
        with ExitStack() as pctx:
            x_pool = pctx.enter_context(tc.tile_pool(name="xh", bufs=1))
            XH = x_pool.tile([P, ND * NQ], BF16, tag="xh")    # [p, db*1024+row]
            w_pool = pctx.enter_context(tc.tile_pool(name="w", bufs=1))
            WK = w_pool.tile([P, ND * D], BF16, tag="wwk")
            WV = w_pool.tile([P, ND * D], BF16, tag="wwv")
            WQ = w_pool.tile([P, ND * D], BF16, tag="wwq")
            # halves so the first K matmuls start before the full loads land
            half = ND * NQ // 2
            nc.sync.dma_start(XH[:, :half], xT[:, :half])
            nc.sync.dma_start(WK[:, :half], wkT[:, :half])
            nc.sync.dma_start(XH[:, half:], xT[:, half:])
            nc.sync.dma_start(WK[:, half:], wkT[:, half:])
            nc.sync.dma_start(WV[:], wvT[:, :])
            nc.sync.dma_start(WQ[:], wqT[:, :])

            stage_pool = pctx.enter_context(tc.tile_pool(name="stage", bufs=1))
            psum_p = pctx.enter_context(
                tc.tile_pool(name="psum_p", bufs=2, space="PSUM", side="right")
            )

            # --- K projection: K^T [o, local k] -> k_own_d ---
            kst = stage_pool.tile([P, ND * NQ], BF16, tag="stage", name="kst")
            for ob in range(NO):
                kps = [psum_p.tile([P, 512], F32, tag="psp", name=f"kps{kc}")
                       for kc in range(2)]
                for d in range(ND):
                    for kc in range(2):  # share the stationary wk slice
                        nc.tensor.matmul(
                            kps[kc][:],
                            WK[:, d * D + ob * P : d * D + (ob + 1) * P],
                            XH[:, d * NQ + kc * 512 : d * NQ + (kc + 1) * 512],
                            start=(d == 0),
                            stop=(d == ND - 1),
                        )
                for kc in range(2):
                    nc.scalar.copy(
                        kst[:, ob * NQ + kc * 512 : ob * NQ + (kc + 1) * 512],
                        kps[kc][:],
                    )
                if ob in (1, 3, 5):
                    nc.scalar.dma_start(
                        k_own_d[:, (ob - 1) * NQ : (ob + 1) * NQ],
                        kst[:, (ob - 1) * NQ : (ob + 1) * NQ],
                    )
            nc.scalar.dma_start(k_own_d[:, 6 * NQ :], kst[:, 6 * NQ :])

            # --- K exchange (V/Q still projecting) ---
            if phase != "nocoll":
                nc.gpsimd.collective_compute(
                    "AllGather",
                    mybir.AluOpType.bypass,
                    replica_groups=GROUPS,
                    ins=[k_own_d.opt()],
                    outs=[k_g.opt()],
                )
            else:
                nc.gpsimd.dma_start(k_g[0:P, :], k_own_d[:, :])
                nc.gpsimd.dma_start(k_g[P : 2 * P, :], k_own_d[:, :])
            # permuted readback into natural key order (uniform on the pair)
            for g in range(8):
                r, lc = next(
                    (r, lc) for r in range(2) for lc in range(4) if GMAP[r][lc] == g
                )
                nc.gpsimd.dma_start(
                    KTb[:, :]
                    .rearrange("p (ob k) -> p ob k", ob=NO)[
                        :, :, g * 256 : (g + 1) * 256
                    ],
                    k_g[r * P : (r + 1) * P, :]
                    .rearrange("p (ob k) -> p ob k", ob=NO)[
                        :, :, lc * 256 : (lc + 1) * 256
                    ],
                )

            # --- V projection: V [local k, o] -> v_own_d ---
            vst = stage_pool.tile([P, 8 * D], BF16, tag="stage", name="vst")
            for sb in range(8):
                vps = [psum_p.tile([P, 512], F32, tag="psp", name=f"vps{oc}")
                       for oc in range(2)]
                for d in range(ND):
                    for oc in range(2):  # share the stationary xh slice
                        nc.tensor.matmul(
                            vps[oc][:],
                            XH[:, d * NQ + sb * P : d * NQ + (sb + 1) * P],
                            WV[:, d * D + oc * 512 : d * D + (oc + 1) * 512],
                            start=(d == 0),
                            stop=(d == ND - 1),
                        )
                for oc in range(2):
                    nc.scalar.copy(
                        vst[:, sb * D + oc * 512 : sb * D + (oc + 1) * 512],
                        vps[oc][:],
                    )
                if sb in (1, 3, 5):
                    nc.scalar.dma_start(
                        v_own_d[:, (sb - 1) * D : (sb + 1) * D],
                        vst[:, (sb - 1) * D : (sb + 1) * D],
                    )
            nc.scalar.dma_start(v_own_d[:, 6 * D :], vst[:, 6 * D :])

            if phase != "nocoll":
                nc.gpsimd.collective_compute(
                    "AllGather",
                    mybir.AluOpType.bypass,
                    replica_groups=GROUPS,
                    ins=[v_own_d.opt()],
                    outs=[v_g.opt()],
                )
            else:
                nc.gpsimd.dma_start(v_g[0:P, :], v_own_d[:, :])
                nc.gpsimd.dma_start(v_g[P : 2 * P, :], v_own_d[:, :])
            for g in range(8):
                r, lc = next(
                    (r, lc) for r in range(2) for lc in range(4) if GMAP[r][lc] == g
                )
                nc.gpsimd.dma_start(
                    Vb[:, 2 * g * D : (2 * g + 2) * D],
                    v_g[r * P : (r + 1) * P, 2 * lc * D : (2 * lc + 2) * D],
                )

            # --- Q projection: QT [o, local q], kept resident ---
            for ob in range(NO):
                qps = [psum_p.tile([P, 512], F32, tag="psp", name=f"qps{qc}")
                       for qc in range(2)]
                for d in range(ND):
                    for qc in range(2):  # share the stationary wq slice
                        nc.tensor.matmul(
                            qps[qc][:],
                            WQ[:, d * D + ob * P : d * D + (ob + 1) * P],
                            XH[:, d * NQ + qc * 512 : d * NQ + (qc + 1) * 512],
                            start=(d == 0),
                            stop=(d == ND - 1),
                        )
                for qc in range(2):
                    nc.scalar.copy(
                        QTb[:, ob * NQ + qc * 512 : ob * NQ + (qc + 1) * 512],
                        qps[qc][:],
                    )

        # ================= attention =================
        if phase == "proj":
            return
        with ExitStack() as actx:
            sctx = ExitStack()
            psum_s = sctx.enter_context(
                tc.tile_pool(name="psum_s", bufs=3, space="PSUM", side="right"))
            psum_o = actx.enter_context(tc.tile_pool(name="psum_o", bufs=3, space="PSUM"))
            psum_z = actx.enter_context(tc.tile_pool(name="psum_z", bufs=2, space="PSUM"))

            for s in range(4):
                cs = S_C[s]
                jlo, jhi = 2 * S_GLO[s], 2 * S_GHI[s]
                mcol = S_MCOL[s]
                q0 = s * 256
                Pt = [pt_pool.tile([P, 256], BF16, tag=f"pt{j}", name=f"pt{s}_{j}")
                      for j in range(cs)]
                Ops = [psum_o.tile([P, 512], F32, tag="pso", name=f"o{s}{i}")
                       for i in range(4)]
                Zps = [psum_z.tile([P, 1], F32, tag="psz", name=f"z{s}{qs}")
                       for qs in range(2)]

                def emit_scores(j):
                    sp = psum_s.tile([P, 256], F32, tag="pss", name=f"sp{s}_{j}")
                    for ob in range(NO):
                        nc.tensor.matmul(
                            sp[:],
                            KTb[:, ob * N + j * P : ob * N + (j + 1) * P],
                            QTb[:, ob * NQ + q0 : ob * NQ + q0 + 256],
                            start=(ob == 0),
                            stop=(ob == NO - 1),
                        )
                    if j < jlo:
                        nc.scalar.activation(Pt[j][:], sp[:],
                                             mybir.ActivationFunctionType.Exp)
                    else:
                        lowd = j < jlo + 2
                        M = mask_pool.tile([P, 256], F32, tag="m", name=f"m{s}_{j}")
                        nc.gpsimd.affine_select(
                            out=M[:],
                            in_=zeros[:] if lowd else m1row[mcol][:],
                            compare_op=mybir.AluOpType.is_ge,
                            fill=mreg[mcol] if lowd else MASK_VAL,
                            base=(jlo if lowd else jhi) * P - j * P,
                            pattern=[[1, 256]],
                            channel_multiplier=-1,
                        )
                        nc.vector.tensor_tensor(sp[:], sp[:], M[:],
                                                mybir.AluOpType.add)
                        nc.scalar.activation(Pt[j][:], sp[:],
                                             mybir.ActivationFunctionType.Exp)

                def emit_av(j):
                    for qs in range(2):
                        cq = QS_C[s][qs]
                        if j >= cq:
                            continue
                        stat = Pt[j][:, qs * P : (qs + 1) * P]
                        for oc in range(2):
                            nc.tensor.matmul(
                                Ops[qs * 2 + oc][:],
                                stat,
                                Vb[:, j * D + oc * 512 : j * D + (oc + 1) * 512],
                                start=(j == 0),
                                stop=(j == cq - 1),
                            )
                        nc.tensor.matmul(
                            Zps[qs][:], stat, ones[:],
                            start=(j == 0), stop=(j == cq - 1),
                        )

                emit_scores(0)
                emit_scores(1)
                emit_scores(2)
                for j in range(cs):
                    emit_av(j)
                    if j + 3 < cs:
                        emit_scores(j + 3)

                O = o_pool.tile([P, N], F32, tag="o", name=f"ot{s}")
                for qs in range(2):
                    rz = stat_pool.tile([P, 1], F32, tag="rz", name=f"rz{s}{qs}")
                    nc.vector.reciprocal(rz[:], Zps[qs][:])
                    for oc in range(2):
                        nc.vector.tensor_scalar_mul(
                            O[:, qs * D + oc * 512 : qs * D + (oc + 1) * 512],
                            Ops[qs * 2 + oc][:],
                            rz[:],
                        )
                nc.scalar.dma_start(out[:, s * N : (s + 1) * N], O[:])
            sctx.close()
def _get_program(iters=1, phase="full"):
    key = ("nc", iters, phase)
    if key not in _CACHE:
        _CACHE[key] = _build_program(iters, phase)
    return _CACHE[key]


def _swizzle(mat):
    """[1024, W] -> [128, 8*W] with col = db*W + c for row db*128+p."""
    r, w = mat.shape
    return np.ascontiguousarray(
        mat.reshape(8, P, w).transpose(1, 0, 2).reshape(P, 8 * w)
    )


def _host_prep(x, Wq, Wk, Wv):
    bf16 = mybir.dt.np(BF16)
    scale = np.float32(1.0 / np.sqrt(np.float32(D)))
    wqT = _swizzle(np.ascontiguousarray((np.asarray(Wq, np.float32) * scale).T)).astype(bf16)
    wkT = _swizzle(np.ascontiguousarray(np.asarray(Wk, np.float32).T)).astype(bf16)
    wvT = _swizzle(np.ascontiguousarray(np.asarray(Wv, np.float32).T)).astype(bf16)
    in_maps = []
    for c in range(8):
        b, h = c // 2, c % 2
        rows = np.concatenate(
            [np.asarray(x[b, g * 256 : (g + 1) * 256], np.float32) for g in GMAP[h]]
        )
        m = np.empty((P, 2), np.float32)
        m[:, 0] = 0.0 if h == 0 else MASK_VAL   # window 0: high core is h=0
        m[:, 1] = MASK_VAL if h == 0 else 0.0   # window 1: high core is h=1
        in_maps.append(
            {
                "xT": _swizzle(np.ascontiguousarray(rows.T)).astype(bf16),
                "wqT": wqT,
                "wkT": wkT,
                "wvT": wvT,
                "m1": m,
            }
        )
    return in_maps


def kernel(x, Wq, Wk, Wv):
    nc = _get_program()
    in_maps = _host_prep(x, Wq, Wk, Wv)
    res = run_bass_kernel_spmd(nc, in_maps, list(range(8)))
    _CACHE["last_results"] = res
    out = np.empty((B, N, D), np.float32)
    for c in range(8):
        b, h = c // 2, c % 2
        o = res.results[c]["out"]  # [128, 8*1024]: col = hs*2048 + qs*1024 + o
        loc = o.reshape(P, 4, 2, D).transpose(1, 2, 0, 3).reshape(NQ, D)
        for i, g in enumerate(GMAP[h]):
            out[b, g * 256 : (g + 1) * 256] = loc[i * 256 : (i + 1) * 256]
    return out

